# revision 17
# baseline (speedup 1.0000x reference)
"""Bass/Trainium2 kernel for nn_LinearMultiheadAttention_75204877353238.

Math: the reference einsums share no indices between the activation and the
weight operands, so the whole module collapses to

    a_h     = sum(q_weights[h])                      (scalar per head)
    c_h     = D * sum(v_weights[h])                  (scalar per head)
    vsum[b,v] = sum_s v[b,s,v]
    g[b,h,s]  = sum_d softmax_s(a_h * q[b,s,d])[s,d]
    t[b,h,s]  = c_h * g[b,h,s]
    out[b,s,v] = max_h t[b,h,s] * vsum[b,v]
               = max(vsum[b,v]*max_h t[b,h,s], vsum[b,v]*min_h t[b,h,s])

k and k_weights are mathematically unused (the k-softmax is summed over its
normalization axis, which gives exactly 1).

Sharding: 8 cores; core c handles batch c//2 and head group c%2 (4 heads).
Host combines the two per-core partial head-maxes per batch with np.maximum.

Per-core pipeline (engine balance):
  DMA   : 16 batched q loads, 16 v loads, 2 weight loads, 16 out stores
  PE    : q transposes (d onto partitions), vsum ones-matvec, t-col matvecs
  ACT   : exp with fused Z row-sum (accum_out), half the PSUM->SBUF copies
  DVE   : min-tree, half the copies, reciprocal, out-stage select-max
  Pool  : max-tree, out-stage multiplies
"""

import numpy as np

import concourse.bacc as bacc
import concourse.bass as bass
import concourse.mybir as mybir
import concourse.tile as tile
from concourse.bass_utils import run_bass_kernel_spmd
from concourse.masks import make_identity

B, S, D, H = 4, 8192, 256, 8
P = 128
NCORES = 8
HPC = H // 2            # heads per core
NCHUNK = S // P         # 64 s-chunks of 128
NB = 4                  # s-chunks per DMA batch
NBATCH = NCHUNK // NB   # 16 DMA batches
ND = D // P             # 2 d-tiles
SH = S // 2             # s-half for eT tiles
F32 = mybir.dt.float32
AF = mybir.ActivationFunctionType
ALU = mybir.AluOpType
AX = mybir.AxisListType
ts = bass.ts

TRACE = False
LAST_RESULTS = None


def _build_nc():
    nc = bacc.Bacc("TRN2", target_bir_lowering=False, debug=False)

    qd = nc.dram_tensor("q", [S, D], F32, kind="ExternalInput")
    vd = nc.dram_tensor("v", [S, D], F32, kind="ExternalInput")
    qwd = nc.dram_tensor("qw", [HPC, D, D], F32, kind="ExternalInput")
    vwd = nc.dram_tensor("vw", [HPC, D, D], F32, kind="ExternalInput")
    outd = nc.dram_tensor("out", [S, D], F32, kind="ExternalOutput")

    with tile.TileContext(nc) as tc:
        _body(nc, tc, qd, vd, qwd, vwd, outd)

    nc.compile()
    return nc


def _body(nc, tc, qd, vd, qwd, vwd, outd):
    qd4 = qd.rearrange("(i n p) d -> i p n d", p=P, n=NB)    # [16,128,4,256]
    vd4 = vd.rearrange("(i n p) d -> i p n d", p=P, n=NB)
    outd4 = outd.rearrange("(i n p) d -> i p n d", p=P, n=NB)

    with (
        tc.tile_pool(name="consts", bufs=1) as consts,
        tc.tile_pool(name="big", bufs=1) as big,
        tc.tile_pool(name="et_pool", bufs=5) as et_pool,
        tc.tile_pool(name="io", bufs=2) as io,
        tc.tile_pool(name="small", bufs=2) as small,
        tc.tile_pool(name="pst", bufs=2, space="PSUM") as pst,
        tc.tile_pool(name="psv", bufs=1, space="PSUM") as psv,
        tc.tile_pool(name="psc", bufs=2, space="PSUM") as psc,
    ):
        identity = consts.tile([P, P], F32)
        make_identity(nc, identity)
        ones_col = consts.tile([P, 1], F32)
        nc.vector.memset(ones_col, 1.0)
        ones_row = consts.tile([1, P], F32)
        nc.vector.memset(ones_row, 1.0)

        # ---- per-head scalars a_h, c_h (replicated across partitions) ----
        def head_scalar_reps(wd, scale, pfx):
            wt = io.tile([P, 2 * HPC, D], F32, tag="wload", bufs=1, name=f"{pfx}wload")
            nc.sync.dma_start(wt, wd.rearrange("h (t p) d -> p (h t) d", p=P))
            reps = []
            for h in range(HPC):
                wsum = small.tile([P, 1], F32, tag="wsum", name=f"{pfx}ws{h}")
                nc.vector.tensor_reduce(wsum, wt[:, 2 * h:2 * h + 2, :],
                                        axis=AX.XY, op=ALU.add)
                wtot_ps = psc.tile([1, 1], F32, tag="tcol", name=f"{pfx}wt{h}")
                nc.tensor.matmul(wtot_ps, wsum, ones_col)
                wtot_sb = small.tile([1, 1], F32, tag="wtot_sb",
                                     name=f"{pfx}wsb{h}")
                nc.vector.tensor_copy(wtot_sb, wtot_ps)
                rep_ps = psc.tile([P, 1], F32, tag="tcol", name=f"{pfx}rp{h}")
                nc.tensor.matmul(rep_ps, ones_row, wtot_sb)
                rep = small.tile([P, 1], F32, tag=f"{pfx}rep{h}", bufs=1,
                                 name=f"{pfx}rep{h}")
                if scale == 1.0:
                    nc.vector.tensor_copy(rep, rep_ps)
                else:
                    nc.scalar.mul(rep, rep_ps, scale)
                reps.append(rep)
            return reps

        a_rep = head_scalar_reps(qwd, 1.0, "a")
        c_rep = head_scalar_reps(vwd, float(D), "c")

        # qT: transposed q, chunk i occupies cols [256*i, 256*(i+1)) as (d0|d1)
        qTt = big.tile([P, NCHUNK * D // P * P], F32, name="qTt")  # [128, 16384]
        qTv = qTt.rearrange("p (i t f) -> p i t f", t=ND, f=P)     # [128,64,2,128]

        maxaccs = [big.tile([P, NB * D], F32, name=f"maxacc{k}")
                   for k in range(2)]
        minaccs = [big.tile([P, NB * D], F32, name=f"minacc{k}")
                   for k in range(2)]

        # ---- q: load, running col-max/min trees (2 parity chains), transpose
        for i in range(NBATCH):
            qt = io.tile([P, NB, D], F32, tag="qload", bufs=4, name=f"qload{i}")
            nc.sync.dma_start(qt, qd4[i])
            qt_flat = qt.rearrange("p n d -> p (n d)")
            k = i % 2
            if i < 2:
                nc.vector.tensor_copy(maxaccs[k], qt_flat)
                nc.vector.tensor_copy(minaccs[k], qt_flat)
            else:
                nc.vector.tensor_tensor(maxaccs[k], maxaccs[k], qt_flat,
                                        op=ALU.max)
                nc.vector.tensor_tensor(minaccs[k], minaccs[k], qt_flat,
                                        op=ALU.min)
            ptt = pst.tile([P, NB * D], F32, tag="ptt", name=f"ptt{i}")
            for n in range(NB):
                for d in range(ND):
                    nc.tensor.transpose(ptt[:, ts(n * ND + d, P)],
                                        qt[:, n, ts(d, P)], identity)
            nc.scalar.copy(qTt[:, ts(i, NB * D)], ptt)

        # ---- finalize q col stats: [128,1024] -> per-d-lane negated max/min ----
        nmax = small.tile([P, D], F32, tag="nmax", bufs=1, name="nmax")
        nmin = small.tile([P, D], F32, tag="nmin", bufs=1, name="nmin")
        nc.vector.tensor_tensor(maxaccs[0], maxaccs[0], maxaccs[1], op=ALU.max)
        nc.vector.tensor_tensor(minaccs[0], minaccs[0], minaccs[1], op=ALU.min)
        nc.vector.tensor_reduce(nmax,
                                maxaccs[0].rearrange("p (n d) -> p d n", n=NB),
                                axis=AX.X, op=ALU.max)
        nc.vector.tensor_reduce(nmin,
                                minaccs[0].rearrange("p (n d) -> p d n", n=NB),
                                axis=AX.X, op=ALU.min)
        nqmax, nqmin = [], []
        for (name, acc, op) in (("nqmax", nmax, ALU.max), ("nqmin", nmin, ALU.min)):
            ptm = pst.tile([P, D], F32, tag="ptt", name=f"ptm_{name}")
            for d in range(ND):
                nc.tensor.transpose(ptm[:, ts(d, P)], acc[:, ts(d, P)], identity)
            cols = []
            for d in range(ND):
                col = small.tile([P, 1], F32, tag=f"{name}{d}", bufs=1,
                                 name=f"{name}{d}")
                nc.vector.tensor_reduce(col, ptm[:, ts(d, P)], axis=AX.X, op=op)
                nc.vector.tensor_scalar_mul(col, col, -1.0)
                cols.append(col)
            (nqmax if name == "nqmax" else nqmin).extend(cols)


        # ---- per head: exp (+fused Z), 1/Z, t columns via PE matvec ----
        tcur = big.tile([P, NCHUNK], F32, name="tcur")
        tmxall = big.tile([P, NCHUNK], F32, name="tmxall")
        tmnall = big.tile([P, NCHUNK], F32, name="tmnall")
        for h in range(HPC):
            negm = []
            for d in range(ND):
                mp = small.tile([P, 1], F32, tag="mp", name=f"mp{h}_{d}")
                nc.vector.tensor_tensor(mp, a_rep[h], nqmax[d], op=ALU.mult)
                mn = small.tile([P, 1], F32, tag="mn", name=f"mn{h}_{d}")
                nc.vector.tensor_tensor(mn, a_rep[h], nqmin[d], op=ALU.mult)
                nm = small.tile([P, 1], F32, tag="negm", name=f"negm{h}_{d}")
                nc.vector.tensor_tensor(nm, mp, mn, op=ALU.min)
                negm.append(nm)

            eT = [[None] * 2 for _ in range(ND)]
            zp = [[None] * 2 for _ in range(ND)]
            for half in range(2):
                for d in range(ND):
                    e = et_pool.tile([P, SH], F32, tag="eT",
                                     name=f"eT{h}_{d}_{half}")
                    z = small.tile([P, 1], F32, tag="zp", bufs=8,
                                   name=f"zp{h}_{d}_{half}")
                    nc.scalar.activation(
                        e.rearrange("p (i f) -> p i f", f=P),
                        qTv[:, 32 * half:32 * (half + 1), d, :],
                        AF.Exp, bias=negm[d], scale=a_rep[h], accum_out=z)
                    eT[d][half] = e
                    zp[d][half] = z
            rc = []
            for d in range(ND):
                z = small.tile([P, 1], F32, tag="zs", name=f"z{h}_{d}")
                nc.vector.tensor_tensor(z, zp[d][0], zp[d][1], op=ALU.add)
                r = small.tile([P, 1], F32, tag="r", name=f"r{h}_{d}")
                nc.vector.reciprocal(r, z)
                rcd = small.tile([P, 1], F32, tag="rc", bufs=4,
                                 name=f"rc{h}_{d}")
                nc.vector.tensor_tensor(rcd, r, c_rep[h], op=ALU.mult)
                rc.append(rcd)

            for j8 in range(NCHUNK // 8):
                tps = psc.tile([P, 8], F32, tag="tcol", name=f"tps{h}_{j8}")
                for jj in range(8):
                    j = j8 * 8 + jj
                    half, jloc = j // 32, j % 32
                    for d in range(ND):
                        nc.tensor.matmul(
                            tps[:, jj:jj + 1],
                            eT[d][half][:, ts(jloc, P)], rc[d],
                            start=(d == 0), stop=(d == ND - 1))
                if h == 0:
                    nc.vector.tensor_copy(tmxall[:, ts(j8, 8)], tps)
                    nc.vector.tensor_copy(tmnall[:, ts(j8, 8)], tps)
                else:
                    nc.vector.tensor_copy(tcur[:, ts(j8, 8)], tps)
                    nc.vector.tensor_tensor(tmxall[:, ts(j8, 8)],
                                            tmxall[:, ts(j8, 8)],
                                            tcur[:, ts(j8, 8)], op=ALU.max)
                    nc.vector.tensor_tensor(tmnall[:, ts(j8, 8)],
                                            tmnall[:, ts(j8, 8)],
                                            tcur[:, ts(j8, 8)], op=ALU.min)

        # ---- v: column sums via ones-matvec accumulation ----
        # (scheduled after the q load/transpose phase: DMA+PE are idle then)
        vs_psum = psv.tile([1, 2 * D], F32, tag="vs", name="vs_psum")
        with tc.tile_wait_until(0.030):
            for i in range(NBATCH):
                vt = io.tile([P, NB, D], F32, tag="vload", name=f"vload{i}")
                nc.sync.dma_start(vt, vd4[i])
                vt_flat = vt.rearrange("p n d -> p (n d)")
                for half in range(2):
                    nc.tensor.matmul(
                        vs_psum, ones_col, vt_flat[:, ts(half, 2 * D)],
                        start=(i == 0 and half == 0),
                        stop=(i == NBATCH - 1 and half == 1),
                    )
        vs_sb = small.tile([1, 2 * D], F32, tag="vs_sb", bufs=1, name="vs_sb")
        nc.vector.tensor_copy(vs_sb, vs_psum)
        vs_row = small.tile([1, D], F32, tag="vs_row", bufs=1, name="vs_row")
        nc.vector.tensor_tensor(vs_row, vs_sb[:, 0:D], vs_sb[:, D:2 * D],
                                op=ALU.add)
        vb_psum = psv.tile([P, D], F32, tag="vs", name="vb_psum")
        nc.tensor.matmul(vb_psum, ones_row, vs_row)
        vsum_b = big.tile([P, D], F32, name="vsum_b")
        nc.vector.tensor_copy(vsum_b, vb_psum)

        # ---- out tiles per 8-chunk group (tmxall/tmnall already final) ----
        for j8 in range(NCHUNK // 8):
            tmx = tmxall
            tmn = tmnall
            for i2 in range(2):
                i = j8 * 2 + i2                 # DMA batch index (4 chunks)
                ot = io.tile([P, NB, D], F32, tag="qload", bufs=4, name=f"osb{i}")
                for n in range(NB):
                    jl = i2 * NB + n            # chunk within j8 group
                    tmp = io.tile([P, D], F32, tag="otmp", bufs=6, name=f"otmp{i}_{n}")
                    j = i * NB + n
                    if j % 2 == 0:
                        nc.gpsimd.tensor_scalar_mul(tmp, vsum_b,
                                                    tmx[:, j:j + 1])
                    else:
                        nc.scalar.mul(tmp, vsum_b, tmx[:, j:j + 1])
                    nc.vector.scalar_tensor_tensor(
                        ot[:, n, :], in0=vsum_b, scalar=tmn[:, j:j + 1],
                        in1=tmp, op0=ALU.mult, op1=ALU.max)
                nc.sync.dma_start(outd4[i], ot)


_NC_CACHE = None


def _get_nc():
    global _NC_CACHE
    if _NC_CACHE is None:
        _NC_CACHE = _build_nc()
    return _NC_CACHE


def kernel(q, k, v, q_weights, k_weights, v_weights):
    global LAST_RESULTS
    q = np.asarray(q, dtype=np.float32)
    v = np.asarray(v, dtype=np.float32)
    q_weights = np.asarray(q_weights, dtype=np.float32)
    v_weights = np.asarray(v_weights, dtype=np.float32)

    nc = _get_nc()
    in_maps = []
    for c in range(NCORES):
        b, hg = c // 2, c % 2
        in_maps.append({
            "q": np.ascontiguousarray(q[b]),
            "v": np.ascontiguousarray(v[b]),
            "qw": np.ascontiguousarray(q_weights[hg * HPC:(hg + 1) * HPC]),
            "vw": np.ascontiguousarray(v_weights[hg * HPC:(hg + 1) * HPC]),
        })

    res = run_bass_kernel_spmd(nc, in_maps, core_ids=list(range(NCORES)),
                               trace=TRACE)
    LAST_RESULTS = res
    outs = [np.asarray(r["out"]) for r in res.results]
    full = np.stack([np.maximum(outs[2 * b], outs[2 * b + 1]) for b in range(B)])
    return full
